# revision 2
# baseline (speedup 1.0000x reference)
"""LinearAttention Trainium2 Bass kernel, v3.

Data-parallel over batch (b=8) across 8 NeuronCores. Per core, per
512-pixel block (C=128, n=16384):

  pass 1:  q_ps[hd,n] = wq^T @ x        (2 MMs N=512, layout A)
           kv_ps[n, k|v] = x_s^T @ w_kv (4 MMs N=512, layout B)
           eqA <- exp(q_ps)   [one scalar ACT [128,1024], raw]
           ek  <- exp(k part) [one scalar ACT [128,1024] strided]
           vt  <- cast(v part) [2 DVE casts; ones cols preset]
           ctx01|ctx23 += ek_s^T @ [v01|1] / [1|v23]  (8 MMs N=129, 1 bank)
           per pair p: sb = A^T @ eqA_p  (blockdiag-ones mask matmul ->
             per-head exp-sums broadcast to all 64 partitions of the head),
           rsI = approx(1/sb) [DVE], eqn = eqA_p * rsI [gpsimd, SBUF*SBUF]
           eqn persists (normalized exp q, layout A).

  fold:    MT_pair = blockdiag(ctx/s_k)^T @ woT_pair (+ rank-1 bias/2 on
           pair01; exact since softmax columns sum to 1)

  pass 2:  fin = MT01^T @ eqn01 + MT23^T @ eqn23  (2 MMs N=512)
           out <- DMA(scalar copy of fin)

All matmul operands bf16; PSUM accumulation fp32.
"""

import numpy as np
import ml_dtypes

import concourse.bass as bass
import concourse.tile as tile
from concourse import bacc, mybir
from concourse.bass_utils import run_bass_kernel_spmd
from concourse.masks import make_identity

F32 = mybir.dt.float32
BF16 = mybir.dt.bfloat16
AF = mybir.ActivationFunctionType

C = 128
N = 16384
INNER = 256
NB = 512
SUB = 4
NBLK = N // NB


def build_nc():
    nc = bacc.Bacc("TRN2", target_bir_lowering=False, debug=False, num_devices=8)

    x = nc.dram_tensor("x", [C, N], BF16, kind="ExternalInput")
    wqT = nc.dram_tensor("wqT", [C, 3 * INNER], BF16, kind="ExternalInput")
    woT = nc.dram_tensor("woT", [INNER, C], BF16, kind="ExternalInput")
    b2 = nc.dram_tensor("b2", [1, C], BF16, kind="ExternalInput")
    am = nc.dram_tensor("am", [C, C], BF16, kind="ExternalInput")
    out = nc.dram_tensor("out", [C, N], F32, kind="ExternalOutput")

    with tile.TileContext(nc) as tc:
        with (
            tc.tile_pool(name="consts", bufs=1) as consts,
            tc.tile_pool(name="eqa", bufs=1) as eqa,
            tc.tile_pool(name="xin", bufs=3) as xin,
            tc.tile_pool(name="eqtp", bufs=2) as eqtp,
            tc.tile_pool(name="ekp", bufs=2) as ekp,
            tc.tile_pool(name="vtp", bufs=1) as vtp,
            tc.tile_pool(name="rsp", bufs=2) as rsp,
            tc.tile_pool(name="small", bufs=2) as small,
            tc.tile_pool(name="osb", bufs=4) as osbp,
        ):
            wq_s = consts.tile([C, 3 * INNER], BF16)
            nc.sync.dma_start(out=wq_s, in_=wqT[:, :])
            wo_s = consts.tile([C, 2, C], BF16)
            nc.sync.dma_start(out=wo_s[:, 0, :], in_=woT[0:128, :])
            nc.sync.dma_start(out=wo_s[:, 1, :], in_=woT[128:256, :])
            b2_s = consts.tile([1, C], BF16)
            nc.sync.dma_start(out=b2_s, in_=b2[:, :])
            am_s = consts.tile([C, C], BF16)
            nc.sync.dma_start(out=am_s, in_=am[:, :])
            ones1 = consts.tile([1, C], BF16)
            nc.gpsimd.memset(ones1, 1.0)
            ident = consts.tile([C, C], BF16)
            make_identity(nc, ident)

            eqn = eqa.tile([C, 2, N], BF16)       # normalized exp(q), layout A
            MT01 = consts.tile([C, C], BF16)
            MT23 = consts.tile([C, C], BF16)

            vt2 = vtp.tile([C, 2, SUB, 258], BF16)
            nc.gpsimd.memset(vt2[:, 0, :, 128:130], 1.0)
            nc.gpsimd.memset(vt2[:, 1, :, 128:130], 1.0)

            with (
                tc.tile_pool(name="qp", bufs=1, space="PSUM") as qp,
                tc.tile_pool(name="kvp", bufs=1, space="PSUM") as kvp,
                tc.tile_pool(name="ctxp", bufs=1, space="PSUM") as ctxp,
                tc.tile_pool(name="sbp", bufs=1, space="PSUM") as sbp,
            ):
                ctx = ctxp.tile([C, 512], F32)
                sbF = sbp.tile([C, 2, NB], F32)

                # software-pipelined emission: producers for block `blk` and
                # consumers for block `blk-1` share an iteration, so every
                # PE instruction's dependencies were issued a full slot
                # earlier and the in-order PE queue never stalls mid-stream.
                eqA_t = [None, None]
                ek_t = [None, None]

                for blk in range(NBLK + 1):
                    eqA = None
                    if blk < NBLK:
                        nsl = slice(blk * NB, (blk + 1) * NB)
                        x_blk = xin.tile([C, NB], BF16, tag="x")
                        nc.sync.dma_start(out=x_blk, in_=x[:, nsl])

                        eqA = eqtp.tile([C, 2, NB], BF16, tag="eqA")
                        # q half 0 (1-bank q psum; half 1 emitted later so the
                        # WAR on the bank sits behind kv/ctx work)
                        q_ps = qp.tile([C, 2, NB // 2], F32, tag="q")
                        nc.tensor.matmul(
                            q_ps[:, 0, :], lhsT=wq_s[:, 0:128],
                            rhs=x_blk[:, 0:256],
                            start=True, stop=True, skip_group_check=True,
                        )
                        nc.tensor.matmul(
                            q_ps[:, 1, :], lhsT=wq_s[:, 128:256],
                            rhs=x_blk[:, 0:256],
                            start=True, stop=True, skip_group_check=True,
                        )
                        nc.scalar.activation(eqA[:, :, 0:256], q_ps, AF.Exp)
                        kv_ps = kvp.tile([C, SUB, NB], F32, tag="kv")
                        for s in range(SUB):
                            nc.tensor.matmul(
                                kv_ps[:, s, :],
                                lhsT=x_blk[:, s * 128:(s + 1) * 128],
                                rhs=wq_s[:, 256:768],
                                start=True, stop=True, skip_group_check=True,
                            )

                    if blk >= 1:
                        # consumers of block blk-1 on the PE stream
                        p = blk - 1
                        pek = ek_t[p % 2]
                        pvt = vt2[:, p % 2]
                        peq = eqA_t[p % 2]
                        pnsl = slice(p * NB, (p + 1) * NB)
                        for s in range(SUB):
                            nc.tensor.matmul(
                                ctx[:, 0:129], lhsT=pek[:, s, 0:128],
                                rhs=pvt[:, s, 0:129],
                                start=(p == 0 and s == 0),
                                stop=(p == NBLK - 1 and s == SUB - 1),
                                skip_group_check=True,
                            )
                            nc.tensor.matmul(
                                ctx[:, 129:258], lhsT=pek[:, s, 128:256],
                                rhs=pvt[:, s, 129:258],
                                start=(p == 0 and s == 0),
                                stop=(p == NBLK - 1 and s == SUB - 1),
                                skip_group_check=True,
                            )
                        # both sb matmuls adjacent (2 banks), one approx op
                        nc.tensor.matmul(
                            sbF[:, 0, :], lhsT=am_s, rhs=peq[:, 0, :],
                            start=True, stop=True, skip_group_check=True,
                        )
                        nc.tensor.matmul(
                            sbF[:, 1, :], lhsT=am_s, rhs=peq[:, 1, :],
                            start=True, stop=True, skip_group_check=True,
                        )
                        rsI = rsp.tile([C, 2, NB], F32, tag="rsI")
                        nc.vector.reciprocal_approx_fast(rsI, sbF)
                        nc.gpsimd.tensor_mul(
                            eqn[:, 0, pnsl], peq[:, 0, :], rsI[:, 0, :]
                        )
                        nc.gpsimd.tensor_mul(
                            eqn[:, 1, pnsl], peq[:, 1, :], rsI[:, 1, :]
                        )

                    if blk < NBLK:
                        # q half 1 reuses the q psum bank after the first exp
                        nc.tensor.matmul(
                            q_ps[:, 0, :], lhsT=wq_s[:, 0:128],
                            rhs=x_blk[:, 256:512],
                            start=True, stop=True, skip_group_check=True,
                        )
                        nc.tensor.matmul(
                            q_ps[:, 1, :], lhsT=wq_s[:, 128:256],
                            rhs=x_blk[:, 256:512],
                            start=True, stop=True, skip_group_check=True,
                        )
                        nc.scalar.activation(eqA[:, :, 256:512], q_ps, AF.Exp)
                        ek = ekp.tile([C, SUB, INNER], BF16, tag="ek")
                        nc.scalar.activation(ek, kv_ps[:, :, 0:256], AF.Exp)
                        eqA_t[blk % 2] = eqA
                        ek_t[blk % 2] = ek

                        vt = vt2[:, blk % 2]
                        nc.vector.tensor_copy(
                            vt[:, :, 0:128], kv_ps[:, :, 256:384]
                        )
                        nc.vector.tensor_copy(
                            vt[:, :, 130:258], kv_ps[:, :, 384:512]
                        )

                # ---- fold ----
                rk01 = small.tile([C, 1], F32, tag="rk01")
                rk23 = small.tile([C, 1], F32, tag="rk23")
                nc.vector.reciprocal(rk01, ctx[:, 128:129])
                nc.vector.reciprocal(rk23, ctx[:, 129:130])
                bd01 = consts.tile([C, C], BF16)
                bd23 = consts.tile([C, C], BF16)
                nc.gpsimd.memset(bd01, 0.0)
                nc.gpsimd.memset(bd23, 0.0)
                nc.vector.tensor_scalar_mul(
                    bd01[0:64, 0:64], ctx[0:64, 0:64], rk01[0:64, 0:1]
                )
                nc.vector.tensor_scalar_mul(
                    bd01[64:128, 64:128], ctx[64:128, 64:128], rk01[64:128, 0:1]
                )
                nc.vector.tensor_scalar_mul(
                    bd23[0:64, 0:64], ctx[0:64, 130:194], rk23[0:64, 0:1]
                )
                nc.vector.tensor_scalar_mul(
                    bd23[64:128, 64:128], ctx[64:128, 194:258], rk23[64:128, 0:1]
                )

            with (
                tc.tile_pool(name="trp", bufs=2, space="PSUM") as trp,
                tc.tile_pool(name="mtp", bufs=2, space="PSUM") as mtp,
            ):
                for pair, bd, mt in ((0, bd01, MT01), (1, bd23, MT23)):
                    tr = trp.tile([C, C], BF16, tag="tr")
                    nc.tensor.transpose(tr, bd, ident)
                    bdt = consts.tile([C, C], BF16, tag=f"bdt{pair}")
                    nc.vector.tensor_copy(bdt, tr)
                    mm = mtp.tile([C, C], F32, tag="mt")
                    nc.tensor.matmul(
                        mm, lhsT=bdt, rhs=wo_s[:, pair, :],
                        start=True, stop=(pair == 1), skip_group_check=True,
                    )
                    if pair == 0:
                        nc.tensor.matmul(
                            mm, lhsT=ones1, rhs=b2_s,
                            start=False, stop=True, skip_group_check=True,
                        )
                    nc.vector.tensor_copy(mt, mm)

            # ---- pass 2 ----
            with tc.tile_pool(name="finp", bufs=4, space="PSUM") as finp:
                for blk in range(NBLK):
                    nsl = slice(blk * NB, (blk + 1) * NB)
                    fin = finp.tile([C, NB], F32, tag="fin")
                    nc.tensor.matmul(
                        fin, lhsT=MT01, rhs=eqn[:, 0, nsl],
                        start=True, stop=False, skip_group_check=True,
                    )
                    nc.tensor.matmul(
                        fin, lhsT=MT23, rhs=eqn[:, 1, nsl],
                        start=False, stop=True, skip_group_check=True,
                    )
                    ob = osbp.tile([C, NB], F32, tag="ob")
                    if blk % 2 == 0:
                        nc.scalar.copy(ob, fin)
                    else:
                        nc.vector.tensor_copy(ob, fin)
                    nc.sync.dma_start(out=out[:, nsl], in_=ob)

    nc.compile()
    return nc


_NC_CACHE = None


def _prep_consts(w_qkv, w_out, b_out):
    bf = ml_dtypes.bfloat16
    wqT = np.ascontiguousarray(np.asarray(w_qkv, dtype=np.float32).T.astype(bf))
    woT = np.ascontiguousarray(np.asarray(w_out, dtype=np.float32).T.astype(bf))
    b2 = np.ascontiguousarray(
        (np.asarray(b_out, dtype=np.float32) / 2.0).reshape(1, C).astype(bf)
    )
    amk = np.zeros((C, C), dtype=np.float32)
    for p in range(C):
        h = p // 64
        amk[p, h * 64:(h + 1) * 64] = 1.0
    amk = np.ascontiguousarray(amk.astype(bf))
    return wqT, woT, b2, amk


def kernel(x, w_qkv, w_out, b_out):
    global _NC_CACHE
    if _NC_CACHE is None:
        _NC_CACHE = build_nc()
    nc = _NC_CACHE

    b = x.shape[0]
    bf = ml_dtypes.bfloat16
    wqT, woT, b2, amk = _prep_consts(w_qkv, w_out, b_out)
    xb = np.asarray(x, dtype=np.float32).reshape(b, C, N).astype(bf)
    in_maps = [
        {"x": np.ascontiguousarray(xb[i]), "wqT": wqT, "woT": woT,
         "b2": b2, "am": amk}
        for i in range(b)
    ]
    res = run_bass_kernel_spmd(nc, in_maps, core_ids=list(range(b)))
    return np.stack(
        [res.results[i]["out"].reshape(C, 128, 128) for i in range(b)]
    ).astype(np.float32)


# revision 3
# speedup vs baseline: 1.5064x; 1.5064x over previous
"""LinearAttention Trainium2 Bass kernel, v3.

Data-parallel over batch (b=8) across 8 NeuronCores. Per core, per
512-pixel block (C=128, n=16384):

  pass 1:  q_ps[hd,n] = wq^T @ x        (2 MMs N=512, layout A)
           kv_ps[n, k|v] = x_s^T @ w_kv (4 MMs N=512, layout B)
           eqA <- exp(q_ps)   [one scalar ACT [128,1024], raw]
           ek  <- exp(k part) [one scalar ACT [128,1024] strided]
           vt  <- cast(v part) [2 DVE casts; ones cols preset]
           ctx01|ctx23 += ek_s^T @ [v01|1] / [1|v23]  (8 MMs N=129, 1 bank)
           per pair p: sb = A^T @ eqA_p  (blockdiag-ones mask matmul ->
             per-head exp-sums broadcast to all 64 partitions of the head),
           rsI = approx(1/sb) [DVE], eqn = eqA_p * rsI [gpsimd, SBUF*SBUF]
           eqn persists (normalized exp q, layout A).

  fold:    MT_pair = blockdiag(ctx/s_k)^T @ woT_pair (+ rank-1 bias/2 on
           pair01; exact since softmax columns sum to 1)

  pass 2:  fin = MT01^T @ eqn01 + MT23^T @ eqn23  (2 MMs N=512)
           out <- DMA(scalar copy of fin)

All matmul operands bf16; PSUM accumulation fp32.
"""

import numpy as np
import ml_dtypes

import concourse.bass as bass
import concourse.tile as tile
from concourse import bacc, mybir
from concourse.bass_utils import run_bass_kernel_spmd
from concourse.masks import make_identity

F32 = mybir.dt.float32
BF16 = mybir.dt.bfloat16
AF = mybir.ActivationFunctionType

C = 128
N = 16384
INNER = 256
NB = 512
SUB = 4
NBLK = N // NB


def build_nc():
    nc = bacc.Bacc("TRN2", target_bir_lowering=False, debug=False, num_devices=8)

    x = nc.dram_tensor("x", [C, N], BF16, kind="ExternalInput")
    wqT = nc.dram_tensor("wqT", [C, 3 * INNER], BF16, kind="ExternalInput")
    woT = nc.dram_tensor("woT", [INNER, C], BF16, kind="ExternalInput")
    b2 = nc.dram_tensor("b2", [1, C], BF16, kind="ExternalInput")
    am = nc.dram_tensor("am", [C, C], BF16, kind="ExternalInput")
    out = nc.dram_tensor("out", [C, N], F32, kind="ExternalOutput")

    with tile.TileContext(nc) as tc:
        with (
            tc.tile_pool(name="consts", bufs=1) as consts,
            tc.tile_pool(name="eqa", bufs=1) as eqa,
            tc.tile_pool(name="xin", bufs=4) as xin,
            tc.tile_pool(name="eqtp", bufs=2) as eqtp,
            tc.tile_pool(name="ekp", bufs=2) as ekp,
            tc.tile_pool(name="vtp", bufs=1) as vtp,
            tc.tile_pool(name="rsp", bufs=2) as rsp,
            tc.tile_pool(name="small", bufs=2) as small,
            tc.tile_pool(name="osb", bufs=4) as osbp,
        ):
            wq_s = consts.tile([C, 3 * INNER], BF16)
            nc.sync.dma_start(out=wq_s, in_=wqT[:, :])
            wo_s = consts.tile([C, 2, C], BF16)
            nc.sync.dma_start(out=wo_s[:, 0, :], in_=woT[0:128, :])
            nc.sync.dma_start(out=wo_s[:, 1, :], in_=woT[128:256, :])
            b2_s = consts.tile([1, C], BF16)
            nc.sync.dma_start(out=b2_s, in_=b2[:, :])
            am_s = consts.tile([C, C], BF16)
            nc.sync.dma_start(out=am_s, in_=am[:, :])
            ones1 = consts.tile([1, C], BF16)
            nc.gpsimd.memset(ones1, 1.0)
            ident = consts.tile([C, C], BF16)
            make_identity(nc, ident)

            eqn = eqa.tile([C, 2, N], BF16)       # normalized exp(q), layout A
            MT01 = consts.tile([C, C], BF16)
            MT23 = consts.tile([C, C], BF16)

            vt2 = vtp.tile([C, 2, SUB, 258], BF16)
            nc.gpsimd.memset(vt2[:, 0, :, 128:130], 1.0)
            nc.gpsimd.memset(vt2[:, 1, :, 128:130], 1.0)

            with (
                tc.tile_pool(name="qp", bufs=1, space="PSUM") as qp,
                tc.tile_pool(name="kvp", bufs=1, space="PSUM") as kvp,
                tc.tile_pool(name="ctxp", bufs=1, space="PSUM") as ctxp,
                tc.tile_pool(name="sbp", bufs=1, space="PSUM") as sbp,
            ):
                ctx = ctxp.tile([C, 512], F32)
                sbF = sbp.tile([C, 2, NB], F32)

                # software-pipelined emission: producers for block `blk` and
                # consumers for block `blk-1` share an iteration, so every
                # PE instruction's dependencies were issued a full slot
                # earlier and the in-order PE queue never stalls mid-stream.
                eqA_t = [None, None]
                ek_t = [None, None]

                for blk in range(NBLK + 1):
                    eqA = None
                    if blk < NBLK:
                        nsl = slice(blk * NB, (blk + 1) * NB)
                        x_blk = xin.tile([C, NB], BF16, tag="x")
                        nc.sync.dma_start(out=x_blk, in_=x[:, nsl])

                        eqA = eqtp.tile([C, 2, NB], BF16, tag="eqA")
                        # q half 0 (1-bank q psum; half 1 emitted later so the
                        # WAR on the bank sits behind kv/ctx work)
                        q_ps = qp.tile([C, 2, NB // 2], F32, tag="q")
                        nc.tensor.matmul(
                            q_ps[:, 0, :], lhsT=wq_s[:, 0:128],
                            rhs=x_blk[:, 0:256],
                            start=True, stop=True, skip_group_check=True,
                        )
                        nc.tensor.matmul(
                            q_ps[:, 1, :], lhsT=wq_s[:, 128:256],
                            rhs=x_blk[:, 0:256],
                            start=True, stop=True, skip_group_check=True,
                        )
                        nc.scalar.activation(eqA[:, :, 0:256], q_ps, AF.Exp)
                        kv_ps = kvp.tile([C, SUB, NB], F32, tag="kv")
                        for s in range(SUB):
                            nc.tensor.matmul(
                                kv_ps[:, s, :],
                                lhsT=x_blk[:, s * 128:(s + 1) * 128],
                                rhs=wq_s[:, 256:768],
                                start=True, stop=True, skip_group_check=True,
                            )

                    if blk >= 1:
                        # consumers of block blk-1 on the PE stream
                        p = blk - 1
                        pek = ek_t[p % 2]
                        pvt = vt2[:, p % 2]
                        peq = eqA_t[p % 2]
                        pnsl = slice(p * NB, (p + 1) * NB)
                        for s in range(SUB):
                            nc.tensor.matmul(
                                ctx[:, 0:129], lhsT=pek[:, s, 0:128],
                                rhs=pvt[:, s, 0:129],
                                start=(p == 0 and s == 0),
                                stop=(p == NBLK - 1 and s == SUB - 1),
                                skip_group_check=True,
                            )
                            nc.tensor.matmul(
                                ctx[:, 129:258], lhsT=pek[:, s, 128:256],
                                rhs=pvt[:, s, 129:258],
                                start=(p == 0 and s == 0),
                                stop=(p == NBLK - 1 and s == SUB - 1),
                                skip_group_check=True,
                            )
                        # both sb matmuls adjacent (2 banks), one approx op
                        nc.tensor.matmul(
                            sbF[:, 0, :], lhsT=am_s, rhs=peq[:, 0, :],
                            start=True, stop=True, skip_group_check=True,
                        )
                        nc.tensor.matmul(
                            sbF[:, 1, :], lhsT=am_s, rhs=peq[:, 1, :],
                            start=True, stop=True, skip_group_check=True,
                        )
                        rsI = rsp.tile([C, 2, NB], F32, tag="rsI")
                        nc.vector.reciprocal_approx_fast(rsI, sbF)
                        nc.gpsimd.tensor_mul(
                            eqn[:, 0, pnsl], peq[:, 0, :], rsI[:, 0, :]
                        )
                        nc.gpsimd.tensor_mul(
                            eqn[:, 1, pnsl], peq[:, 1, :], rsI[:, 1, :]
                        )

                    if blk < NBLK:
                        # q half 1 reuses the q psum bank after the first exp
                        nc.tensor.matmul(
                            q_ps[:, 0, :], lhsT=wq_s[:, 0:128],
                            rhs=x_blk[:, 256:512],
                            start=True, stop=True, skip_group_check=True,
                        )
                        nc.tensor.matmul(
                            q_ps[:, 1, :], lhsT=wq_s[:, 128:256],
                            rhs=x_blk[:, 256:512],
                            start=True, stop=True, skip_group_check=True,
                        )
                        nc.scalar.activation(eqA[:, :, 256:512], q_ps, AF.Exp)
                        ek = ekp.tile([C, SUB, INNER], BF16, tag="ek")
                        nc.scalar.activation(ek, kv_ps[:, :, 0:256], AF.Exp)
                        eqA_t[blk % 2] = eqA
                        ek_t[blk % 2] = ek

                        vt = vt2[:, blk % 2]
                        nc.vector.tensor_copy(
                            vt[:, :, 0:128], kv_ps[:, :, 256:384]
                        )
                        nc.vector.tensor_copy(
                            vt[:, :, 130:258], kv_ps[:, :, 384:512]
                        )

                # ---- fold ----
                rk01 = small.tile([C, 1], F32, tag="rk01")
                rk23 = small.tile([C, 1], F32, tag="rk23")
                nc.vector.reciprocal(rk01, ctx[:, 128:129])
                nc.vector.reciprocal(rk23, ctx[:, 129:130])
                bd01 = consts.tile([C, C], BF16)
                bd23 = consts.tile([C, C], BF16)
                nc.gpsimd.memset(bd01, 0.0)
                nc.gpsimd.memset(bd23, 0.0)
                nc.vector.tensor_scalar_mul(
                    bd01[0:64, 0:64], ctx[0:64, 0:64], rk01[0:64, 0:1]
                )
                nc.vector.tensor_scalar_mul(
                    bd01[64:128, 64:128], ctx[64:128, 64:128], rk01[64:128, 0:1]
                )
                nc.vector.tensor_scalar_mul(
                    bd23[0:64, 0:64], ctx[0:64, 130:194], rk23[0:64, 0:1]
                )
                nc.vector.tensor_scalar_mul(
                    bd23[64:128, 64:128], ctx[64:128, 194:258], rk23[64:128, 0:1]
                )

            with (
                tc.tile_pool(name="trp", bufs=2, space="PSUM") as trp,
                tc.tile_pool(name="mtp", bufs=2, space="PSUM") as mtp,
            ):
                for pair, bd, mt in ((0, bd01, MT01), (1, bd23, MT23)):
                    tr = trp.tile([C, C], BF16, tag="tr")
                    nc.tensor.transpose(tr, bd, ident)
                    bdt = consts.tile([C, C], BF16, tag=f"bdt{pair}")
                    nc.vector.tensor_copy(bdt, tr)
                    mm = mtp.tile([C, C], F32, tag="mt")
                    nc.tensor.matmul(
                        mm, lhsT=bdt, rhs=wo_s[:, pair, :],
                        start=True, stop=(pair == 1), skip_group_check=True,
                    )
                    if pair == 0:
                        nc.tensor.matmul(
                            mm, lhsT=ones1, rhs=b2_s,
                            start=False, stop=True, skip_group_check=True,
                        )
                    nc.vector.tensor_copy(mt, mm)

            # ---- pass 2 ----
            with tc.tile_pool(name="finp", bufs=4, space="PSUM") as finp:
                for blk in range(NBLK):
                    nsl = slice(blk * NB, (blk + 1) * NB)
                    fin = finp.tile([C, NB], F32, tag="fin")
                    nc.tensor.matmul(
                        fin, lhsT=MT01, rhs=eqn[:, 0, nsl],
                        start=True, stop=False, skip_group_check=True,
                    )
                    nc.tensor.matmul(
                        fin, lhsT=MT23, rhs=eqn[:, 1, nsl],
                        start=False, stop=True, skip_group_check=True,
                    )
                    ob = osbp.tile([C, NB], F32, tag="ob")
                    nc.scalar.copy(ob[:, 0:256], fin[:, 0:256])
                    nc.vector.tensor_copy(ob[:, 256:512], fin[:, 256:512])
                    if blk % 2 == 0:
                        nc.sync.dma_start(out=out[:, nsl], in_=ob)
                    else:
                        nc.scalar.dma_start(out=out[:, nsl], in_=ob)

    nc.compile()
    return nc


_NC_CACHE = None


def _prep_consts(w_qkv, w_out, b_out):
    bf = ml_dtypes.bfloat16
    wqT = np.ascontiguousarray(np.asarray(w_qkv, dtype=np.float32).T.astype(bf))
    woT = np.ascontiguousarray(np.asarray(w_out, dtype=np.float32).T.astype(bf))
    b2 = np.ascontiguousarray(
        (np.asarray(b_out, dtype=np.float32) / 2.0).reshape(1, C).astype(bf)
    )
    amk = np.zeros((C, C), dtype=np.float32)
    for p in range(C):
        h = p // 64
        amk[p, h * 64:(h + 1) * 64] = 1.0
    amk = np.ascontiguousarray(amk.astype(bf))
    return wqT, woT, b2, amk


def kernel(x, w_qkv, w_out, b_out):
    global _NC_CACHE
    if _NC_CACHE is None:
        _NC_CACHE = build_nc()
    nc = _NC_CACHE

    b = x.shape[0]
    bf = ml_dtypes.bfloat16
    wqT, woT, b2, amk = _prep_consts(w_qkv, w_out, b_out)
    xb = np.asarray(x, dtype=np.float32).reshape(b, C, N).astype(bf)
    in_maps = [
        {"x": np.ascontiguousarray(xb[i]), "wqT": wqT, "woT": woT,
         "b2": b2, "am": amk}
        for i in range(b)
    ]
    res = run_bass_kernel_spmd(nc, in_maps, core_ids=list(range(b)))
    return np.stack(
        [res.results[i]["out"].reshape(C, 128, 128) for i in range(b)]
    ).astype(np.float32)


# revision 5
# speedup vs baseline: 1.5203x; 1.0093x over previous
"""LinearAttention Trainium2 Bass kernel, v3.

Data-parallel over batch (b=8) across 8 NeuronCores. Per core, per
512-pixel block (C=128, n=16384):

  pass 1:  q_ps[hd,n] = wq^T @ x        (2 MMs N=512, layout A)
           kv_ps[n, k|v] = x_s^T @ w_kv (4 MMs N=512, layout B)
           eqA <- exp(q_ps)   [one scalar ACT [128,1024], raw]
           ek  <- exp(k part) [one scalar ACT [128,1024] strided]
           vt  <- cast(v part) [2 DVE casts; ones cols preset]
           ctx01|ctx23 += ek_s^T @ [v01|1] / [1|v23]  (8 MMs N=129, 1 bank)
           per pair p: sb = A^T @ eqA_p  (blockdiag-ones mask matmul ->
             per-head exp-sums broadcast to all 64 partitions of the head),
           rsI = approx(1/sb) [DVE], eqn = eqA_p * rsI [gpsimd, SBUF*SBUF]
           eqn persists (normalized exp q, layout A).

  fold:    MT_pair = blockdiag(ctx/s_k)^T @ woT_pair (+ rank-1 bias/2 on
           pair01; exact since softmax columns sum to 1)

  pass 2:  fin = MT01^T @ eqn01 + MT23^T @ eqn23  (2 MMs N=512)
           out <- DMA(scalar copy of fin)

All matmul operands bf16; PSUM accumulation fp32.
"""

import numpy as np
import ml_dtypes

import concourse.bass as bass
import concourse.tile as tile
from concourse import bacc, mybir
from concourse.bass_utils import run_bass_kernel_spmd
from concourse.masks import make_identity

F32 = mybir.dt.float32
BF16 = mybir.dt.bfloat16
AF = mybir.ActivationFunctionType

C = 128
N = 16384
INNER = 256
NB = 512
SUB = 4
NBLK = N // NB


def build_nc():
    nc = bacc.Bacc("TRN2", target_bir_lowering=False, debug=False, num_devices=8)

    x = nc.dram_tensor("x", [C, N], BF16, kind="ExternalInput")
    wqT = nc.dram_tensor("wqT", [C, 3 * INNER], BF16, kind="ExternalInput")
    woT = nc.dram_tensor("woT", [INNER, C], BF16, kind="ExternalInput")
    b2 = nc.dram_tensor("b2", [1, C], BF16, kind="ExternalInput")
    am = nc.dram_tensor("am", [C, C], BF16, kind="ExternalInput")
    out = nc.dram_tensor("out", [C, N], F32, kind="ExternalOutput")

    with tile.TileContext(nc) as tc:
        with (
            tc.tile_pool(name="consts", bufs=1) as consts,
            tc.tile_pool(name="eqa", bufs=1) as eqa,
            tc.tile_pool(name="xin", bufs=3) as xin,
            tc.tile_pool(name="eqtp", bufs=2) as eqtp,
            tc.tile_pool(name="ekp", bufs=2) as ekp,
            tc.tile_pool(name="vtp", bufs=1) as vtp,
            tc.tile_pool(name="rsp", bufs=2) as rsp,
            tc.tile_pool(name="small", bufs=2) as small,
            tc.tile_pool(name="osb", bufs=4) as osbp,
        ):
            wq_s = consts.tile([C, 3 * INNER], BF16)
            nc.sync.dma_start(out=wq_s, in_=wqT[:, :])
            wo_s = consts.tile([C, 2, C], BF16)
            nc.sync.dma_start(out=wo_s[:, 0, :], in_=woT[0:128, :])
            nc.sync.dma_start(out=wo_s[:, 1, :], in_=woT[128:256, :])
            b2_s = consts.tile([1, C], BF16)
            nc.sync.dma_start(out=b2_s, in_=b2[:, :])
            am_s = consts.tile([C, C], BF16)
            nc.sync.dma_start(out=am_s, in_=am[:, :])
            ones1 = consts.tile([1, C], BF16)
            nc.gpsimd.memset(ones1, 1.0)
            ident = consts.tile([C, C], BF16)
            make_identity(nc, ident)

            eqn = eqa.tile([C, 2, N], BF16)       # normalized exp(q), layout A
            MT01 = consts.tile([C, C], BF16)
            MT23 = consts.tile([C, C], BF16)

            vt2 = vtp.tile([C, 2, SUB, 258], BF16)
            nc.gpsimd.memset(vt2[:, 0, :, 128:130], 1.0)
            nc.gpsimd.memset(vt2[:, 1, :, 128:130], 1.0)

            with (
                tc.tile_pool(name="qp", bufs=1, space="PSUM") as qp,
                tc.tile_pool(name="kvp", bufs=1, space="PSUM") as kvp,
                tc.tile_pool(name="ctxp", bufs=1, space="PSUM") as ctxp,
                tc.tile_pool(name="sbp", bufs=1, space="PSUM") as sbp,
            ):
                ctx = ctxp.tile([C, 512], F32)
                sbF = sbp.tile([C, 2, NB], F32)

                # software-pipelined emission: producers for block `blk` and
                # consumers for block `blk-1` share an iteration, so every
                # PE instruction's dependencies were issued a full slot
                # earlier and the in-order PE queue never stalls mid-stream.
                eqA_t = [None, None]
                ek_t = [None, None]

                for blk in range(NBLK + 1):
                    eqA = None
                    if blk < NBLK:
                        nsl = slice(blk * NB, (blk + 1) * NB)
                        x_blk = xin.tile([C, NB], BF16, tag="x")
                        nc.sync.dma_start(out=x_blk, in_=x[:, nsl])

                        eqA = eqtp.tile([C, 2, NB], BF16, tag="eqA")
                        # q half 0 (1-bank q psum; half 1 emitted later so the
                        # WAR on the bank sits behind kv/ctx work)
                        q_ps = qp.tile([C, 2, NB // 2], F32, tag="q")
                        nc.tensor.matmul(
                            q_ps[:, 0, :], lhsT=wq_s[:, 0:128],
                            rhs=x_blk[:, 0:256],
                            start=True, stop=True, skip_group_check=True,
                        )
                        nc.tensor.matmul(
                            q_ps[:, 1, :], lhsT=wq_s[:, 128:256],
                            rhs=x_blk[:, 0:256],
                            start=True, stop=True, skip_group_check=True,
                        )
                        nc.scalar.activation(eqA[:, :, 0:256], q_ps, AF.Exp)
                        nc.tensor.matmul(
                            q_ps[:, 0, :], lhsT=wq_s[:, 0:128],
                            rhs=x_blk[:, 256:512],
                            start=True, stop=True, skip_group_check=True,
                        )
                        nc.tensor.matmul(
                            q_ps[:, 1, :], lhsT=wq_s[:, 128:256],
                            rhs=x_blk[:, 256:512],
                            start=True, stop=True, skip_group_check=True,
                        )
                        nc.scalar.activation(eqA[:, :, 256:512], q_ps, AF.Exp)
                        kv_ps = kvp.tile([C, SUB, NB], F32, tag="kv")
                        for s in range(SUB):
                            nc.tensor.matmul(
                                kv_ps[:, s, :],
                                lhsT=x_blk[:, s * 128:(s + 1) * 128],
                                rhs=wq_s[:, 256:768],
                                start=True, stop=True, skip_group_check=True,
                            )

                    if blk >= 1:
                        # consumers of block blk-1 on the PE stream
                        p = blk - 1
                        pek = ek_t[p % 2]
                        pvt = vt2[:, p % 2]
                        peq = eqA_t[p % 2]
                        pnsl = slice(p * NB, (p + 1) * NB)
                        for s in range(SUB):
                            nc.tensor.matmul(
                                ctx[:, 0:129], lhsT=pek[:, s, 0:128],
                                rhs=pvt[:, s, 0:129],
                                start=(p == 0 and s == 0),
                                stop=(p == NBLK - 1 and s == SUB - 1),
                                skip_group_check=True,
                            )
                            nc.tensor.matmul(
                                ctx[:, 129:258], lhsT=pek[:, s, 128:256],
                                rhs=pvt[:, s, 129:258],
                                start=(p == 0 and s == 0),
                                stop=(p == NBLK - 1 and s == SUB - 1),
                                skip_group_check=True,
                            )
                        # both sb matmuls adjacent (2 banks), one approx op
                        nc.tensor.matmul(
                            sbF[:, 0, :], lhsT=am_s, rhs=peq[:, 0, :],
                            start=True, stop=True, skip_group_check=True,
                        )
                        nc.tensor.matmul(
                            sbF[:, 1, :], lhsT=am_s, rhs=peq[:, 1, :],
                            start=True, stop=True, skip_group_check=True,
                        )
                        rsI = rsp.tile([C, 2, NB], F32, tag="rsI")
                        nc.vector.reciprocal_approx_fast(rsI, sbF)
                        nc.gpsimd.tensor_mul(
                            eqn[:, 0, pnsl], peq[:, 0, :], rsI[:, 0, :]
                        )
                        nc.gpsimd.tensor_mul(
                            eqn[:, 1, pnsl], peq[:, 1, :], rsI[:, 1, :]
                        )

                    if blk < NBLK:
                        ek = ekp.tile([C, SUB, INNER], BF16, tag="ek")
                        nc.scalar.activation(ek, kv_ps[:, :, 0:256], AF.Exp)
                        eqA_t[blk % 2] = eqA
                        ek_t[blk % 2] = ek

                        vt = vt2[:, blk % 2]
                        nc.vector.tensor_copy(
                            vt[:, :, 0:128], kv_ps[:, :, 256:384]
                        )
                        nc.vector.tensor_copy(
                            vt[:, :, 130:258], kv_ps[:, :, 384:512]
                        )

                # ---- fold ----
                rk01 = small.tile([C, 1], F32, tag="rk01")
                rk23 = small.tile([C, 1], F32, tag="rk23")
                nc.vector.reciprocal(rk01, ctx[:, 128:129])
                nc.vector.reciprocal(rk23, ctx[:, 129:130])
                bd01 = consts.tile([C, C], BF16)
                bd23 = consts.tile([C, C], BF16)
                nc.gpsimd.memset(bd01, 0.0)
                nc.gpsimd.memset(bd23, 0.0)
                nc.vector.tensor_scalar_mul(
                    bd01[0:64, 0:64], ctx[0:64, 0:64], rk01[0:64, 0:1]
                )
                nc.vector.tensor_scalar_mul(
                    bd01[64:128, 64:128], ctx[64:128, 64:128], rk01[64:128, 0:1]
                )
                nc.vector.tensor_scalar_mul(
                    bd23[0:64, 0:64], ctx[0:64, 130:194], rk23[0:64, 0:1]
                )
                nc.vector.tensor_scalar_mul(
                    bd23[64:128, 64:128], ctx[64:128, 194:258], rk23[64:128, 0:1]
                )

            with (
                tc.tile_pool(name="trp", bufs=2, space="PSUM") as trp,
                tc.tile_pool(name="mtp", bufs=2, space="PSUM") as mtp,
            ):
                for pair, bd, mt in ((0, bd01, MT01), (1, bd23, MT23)):
                    tr = trp.tile([C, C], BF16, tag="tr")
                    nc.tensor.transpose(tr, bd, ident)
                    bdt = consts.tile([C, C], BF16, tag=f"bdt{pair}")
                    nc.vector.tensor_copy(bdt, tr)
                    mm = mtp.tile([C, C], F32, tag="mt")
                    nc.tensor.matmul(
                        mm, lhsT=bdt, rhs=wo_s[:, pair, :],
                        start=True, stop=(pair == 1), skip_group_check=True,
                    )
                    if pair == 0:
                        nc.tensor.matmul(
                            mm, lhsT=ones1, rhs=b2_s,
                            start=False, stop=True, skip_group_check=True,
                        )
                    nc.vector.tensor_copy(mt, mm)

            # ---- pass 2 ----
            with tc.tile_pool(name="finp", bufs=4, space="PSUM") as finp:
                for blk in range(NBLK):
                    nsl = slice(blk * NB, (blk + 1) * NB)
                    fin = finp.tile([C, NB], F32, tag="fin")
                    nc.tensor.matmul(
                        fin, lhsT=MT01, rhs=eqn[:, 0, nsl],
                        start=True, stop=False, skip_group_check=True,
                    )
                    nc.tensor.matmul(
                        fin, lhsT=MT23, rhs=eqn[:, 1, nsl],
                        start=False, stop=True, skip_group_check=True,
                    )
                    ob = osbp.tile([C, NB], F32, tag="ob")
                    if blk % 2 == 0:
                        nc.scalar.copy(ob, fin)
                    else:
                        nc.vector.tensor_copy(ob, fin)
                    nc.sync.dma_start(out=out[:, nsl], in_=ob)

    nc.compile()
    return nc


_NC_CACHE = None


def _prep_consts(w_qkv, w_out, b_out):
    bf = ml_dtypes.bfloat16
    wqT = np.ascontiguousarray(np.asarray(w_qkv, dtype=np.float32).T.astype(bf))
    woT = np.ascontiguousarray(np.asarray(w_out, dtype=np.float32).T.astype(bf))
    b2 = np.ascontiguousarray(
        (np.asarray(b_out, dtype=np.float32) / 2.0).reshape(1, C).astype(bf)
    )
    amk = np.zeros((C, C), dtype=np.float32)
    for p in range(C):
        h = p // 64
        amk[p, h * 64:(h + 1) * 64] = 1.0
    amk = np.ascontiguousarray(amk.astype(bf))
    return wqT, woT, b2, amk


def kernel(x, w_qkv, w_out, b_out):
    global _NC_CACHE
    if _NC_CACHE is None:
        _NC_CACHE = build_nc()
    nc = _NC_CACHE

    b = x.shape[0]
    bf = ml_dtypes.bfloat16
    wqT, woT, b2, amk = _prep_consts(w_qkv, w_out, b_out)
    xb = np.asarray(x, dtype=np.float32).reshape(b, C, N).astype(bf)
    in_maps = [
        {"x": np.ascontiguousarray(xb[i]), "wqT": wqT, "woT": woT,
         "b2": b2, "am": amk}
        for i in range(b)
    ]
    res = run_bass_kernel_spmd(nc, in_maps, core_ids=list(range(b)))
    return np.stack(
        [res.results[i]["out"].reshape(C, 128, 128) for i in range(b)]
    ).astype(np.float32)


# revision 7
# speedup vs baseline: 1.6265x; 1.0698x over previous
"""LinearAttention Trainium2 Bass kernel, v3.

Data-parallel over batch (b=8) across 8 NeuronCores. Per core, per
512-pixel block (C=128, n=16384):

  pass 1:  q_ps[hd,n] = wq^T @ x        (2 MMs N=512, layout A)
           kv_ps[n, k|v] = x_s^T @ w_kv (4 MMs N=512, layout B)
           eqA <- exp(q_ps)   [one scalar ACT [128,1024], raw]
           ek  <- exp(k part) [one scalar ACT [128,1024] strided]
           vt  <- cast(v part) [2 DVE casts; ones cols preset]
           ctx01|ctx23 += ek_s^T @ [v01|1] / [1|v23]  (8 MMs N=129, 1 bank)
           per pair p: sb = A^T @ eqA_p  (blockdiag-ones mask matmul ->
             per-head exp-sums broadcast to all 64 partitions of the head),
           rsI = approx(1/sb) [DVE], eqn = eqA_p * rsI [gpsimd, SBUF*SBUF]
           eqn persists (normalized exp q, layout A).

  fold:    MT_pair = blockdiag(ctx/s_k)^T @ woT_pair (+ rank-1 bias/2 on
           pair01; exact since softmax columns sum to 1)

  pass 2:  fin = MT01^T @ eqn01 + MT23^T @ eqn23  (2 MMs N=512)
           out <- DMA(scalar copy of fin)

All matmul operands bf16; PSUM accumulation fp32.
"""

import numpy as np
import ml_dtypes

import concourse.bass as bass
import concourse.tile as tile
from concourse import bacc, mybir
from concourse.bass_utils import run_bass_kernel_spmd
from concourse.masks import make_identity

F32 = mybir.dt.float32
BF16 = mybir.dt.bfloat16
AF = mybir.ActivationFunctionType

C = 128
N = 16384
INNER = 256
NB = 512
SUB = 4
NBLK = N // NB


def build_nc():
    nc = bacc.Bacc("TRN2", target_bir_lowering=False, debug=False, num_devices=8)

    x = nc.dram_tensor("x", [C, N], BF16, kind="ExternalInput")
    wqT = nc.dram_tensor("wqT", [C, 3 * INNER], BF16, kind="ExternalInput")
    woT = nc.dram_tensor("woT", [INNER, C], BF16, kind="ExternalInput")
    b2 = nc.dram_tensor("b2", [1, C], BF16, kind="ExternalInput")
    am = nc.dram_tensor("am", [C, C], BF16, kind="ExternalInput")
    out = nc.dram_tensor("out", [C, N], F32, kind="ExternalOutput")

    with tile.TileContext(nc) as tc:
        with (
            tc.tile_pool(name="consts", bufs=1) as consts,
            tc.tile_pool(name="eqa", bufs=1) as eqa,
            tc.tile_pool(name="xin", bufs=3) as xin,
            tc.tile_pool(name="eqtp", bufs=2) as eqtp,
            tc.tile_pool(name="ekp", bufs=2) as ekp,
            tc.tile_pool(name="vtp", bufs=1) as vtp,
            tc.tile_pool(name="rsp", bufs=2) as rsp,
            tc.tile_pool(name="small", bufs=2) as small,
            tc.tile_pool(name="osb", bufs=4) as osbp,
        ):
            wq_s = consts.tile([C, 3 * INNER], BF16)
            nc.sync.dma_start(out=wq_s, in_=wqT[:, :])
            wo_s = consts.tile([C, 2, C], BF16)
            nc.sync.dma_start(out=wo_s[:, 0, :], in_=woT[0:128, :])
            nc.sync.dma_start(out=wo_s[:, 1, :], in_=woT[128:256, :])
            b2_s = consts.tile([1, C], BF16)
            nc.sync.dma_start(out=b2_s, in_=b2[:, :])
            am_s = consts.tile([C, C], BF16)
            nc.sync.dma_start(out=am_s, in_=am[:, :])
            ones1 = consts.tile([1, C], BF16)
            nc.gpsimd.memset(ones1, 1.0)
            ident = consts.tile([C, C], BF16)
            make_identity(nc, ident)

            eqn = eqa.tile([C, 2, N], BF16)       # normalized exp(q), layout A
            MT01 = consts.tile([C, C], BF16)
            MT23 = consts.tile([C, C], BF16)

            vt2 = vtp.tile([C, 2, SUB, 258], BF16)
            nc.gpsimd.memset(vt2[:, 0, :, 128:130], 1.0)
            nc.gpsimd.memset(vt2[:, 1, :, 128:130], 1.0)

            with (
                tc.tile_pool(name="qp", bufs=1, space="PSUM") as qp,
                tc.tile_pool(name="kvp", bufs=1, space="PSUM") as kvp,
                tc.tile_pool(name="ctxp", bufs=1, space="PSUM") as ctxp,
                tc.tile_pool(name="sbp", bufs=1, space="PSUM") as sbp,
            ):
                ctx = ctxp.tile([C, 512], F32)
                sbF = sbp.tile([C, 2, NB], F32)

                # software-pipelined emission: producers for block `blk` and
                # consumers for block `blk-1` share an iteration, so every
                # PE instruction's dependencies were issued a full slot
                # earlier and the in-order PE queue never stalls mid-stream.
                eqA_t = [None, None]
                ek_t = [None, None]

                for blk in range(NBLK + 1):
                    eqA = None
                    if blk < NBLK:
                        nsl = slice(blk * NB, (blk + 1) * NB)
                        x_blk = xin.tile([C, NB], BF16, tag="x")
                        nc.sync.dma_start(out=x_blk, in_=x[:, nsl])

                        eqA = eqtp.tile([C, 2, NB], BF16, tag="eqA")
                        # q half 0 (1-bank q psum; half 1 emitted later so the
                        # WAR on the bank sits behind kv/ctx work)
                        q_ps = qp.tile([C, 2, NB // 2], F32, tag="q")
                        nc.tensor.matmul(
                            q_ps[:, 0, :], lhsT=wq_s[:, 0:128],
                            rhs=x_blk[:, 0:256],
                            start=True, stop=True, skip_group_check=True,
                        )
                        nc.tensor.matmul(
                            q_ps[:, 1, :], lhsT=wq_s[:, 128:256],
                            rhs=x_blk[:, 0:256],
                            start=True, stop=True, skip_group_check=True,
                        )
                        nc.scalar.activation(eqA[:, :, 0:256], q_ps, AF.Exp)
                        kv_ps = kvp.tile([C, SUB, NB], F32, tag="kv")
                        for s in range(SUB):
                            nc.tensor.matmul(
                                kv_ps[:, s, :],
                                lhsT=x_blk[:, s * 128:(s + 1) * 128],
                                rhs=wq_s[:, 256:768],
                                start=True, stop=True, skip_group_check=True,
                            )

                    if blk >= 1:
                        # consumers of block blk-1 on the PE stream
                        p = blk - 1
                        pek = ek_t[p % 2]
                        pvt = vt2[:, p % 2]
                        peq = eqA_t[p % 2]
                        pnsl = slice(p * NB, (p + 1) * NB)
                        for s in range(SUB):
                            nc.tensor.matmul(
                                ctx[:, 0:129], lhsT=pek[:, s, 0:128],
                                rhs=pvt[:, s, 0:129],
                                start=(p == 0 and s == 0),
                                stop=(p == NBLK - 1 and s == SUB - 1),
                                skip_group_check=True,
                            )
                            nc.tensor.matmul(
                                ctx[:, 129:258], lhsT=pek[:, s, 128:256],
                                rhs=pvt[:, s, 129:258],
                                start=(p == 0 and s == 0),
                                stop=(p == NBLK - 1 and s == SUB - 1),
                                skip_group_check=True,
                            )
                        # both sb matmuls adjacent (2 banks), one approx op
                        nc.tensor.matmul(
                            sbF[:, 0, :], lhsT=am_s, rhs=peq[:, 0, :],
                            start=True, stop=True, skip_group_check=True,
                        )
                        nc.tensor.matmul(
                            sbF[:, 1, :], lhsT=am_s, rhs=peq[:, 1, :],
                            start=True, stop=True, skip_group_check=True,
                        )
                        rsI = rsp.tile([C, 2, NB], F32, tag="rsI")
                        nc.vector.reciprocal_approx_fast(rsI, sbF)
                        nc.gpsimd.tensor_mul(
                            eqn[:, 0, pnsl], peq[:, 0, :], rsI[:, 0, :]
                        )
                        nc.gpsimd.tensor_mul(
                            eqn[:, 1, pnsl], peq[:, 1, :], rsI[:, 1, :]
                        )

                    if blk < NBLK:
                        # q half 1 reuses the q psum bank after the first exp
                        nc.tensor.matmul(
                            q_ps[:, 0, :], lhsT=wq_s[:, 0:128],
                            rhs=x_blk[:, 256:512],
                            start=True, stop=True, skip_group_check=True,
                        )
                        nc.tensor.matmul(
                            q_ps[:, 1, :], lhsT=wq_s[:, 128:256],
                            rhs=x_blk[:, 256:512],
                            start=True, stop=True, skip_group_check=True,
                        )
                        nc.scalar.activation(eqA[:, :, 256:512], q_ps, AF.Exp)
                        ek = ekp.tile([C, SUB, INNER], BF16, tag="ek")
                        nc.scalar.activation(ek, kv_ps[:, :, 0:256], AF.Exp)
                        eqA_t[blk % 2] = eqA
                        ek_t[blk % 2] = ek

                        vt = vt2[:, blk % 2]
                        vo = vt[:, :, 0:128]
                        vout = bass.AP(
                            tensor=vo.tensor, offset=vo.offset,
                            ap=[list(vo.ap[0]), list(vo.ap[1]), [130, 2],
                                [1, 128]],
                        )
                        vi = kv_ps[:, :, 256:384]
                        vin = bass.AP(
                            tensor=vi.tensor, offset=vi.offset,
                            ap=[list(vi.ap[0]), list(vi.ap[1]), [128, 2],
                                [1, 128]],
                        )
                        nc.vector.tensor_copy(vout, vin)

                # ---- fold ----
                rk01 = small.tile([C, 1], F32, tag="rk01")
                rk23 = small.tile([C, 1], F32, tag="rk23")
                nc.vector.reciprocal(rk01, ctx[:, 128:129])
                nc.vector.reciprocal(rk23, ctx[:, 129:130])
                bd01 = consts.tile([C, C], BF16)
                bd23 = consts.tile([C, C], BF16)
                nc.gpsimd.memset(bd01, 0.0)
                nc.gpsimd.memset(bd23, 0.0)
                nc.vector.tensor_scalar_mul(
                    bd01[0:64, 0:64], ctx[0:64, 0:64], rk01[0:64, 0:1]
                )
                nc.vector.tensor_scalar_mul(
                    bd01[64:128, 64:128], ctx[64:128, 64:128], rk01[64:128, 0:1]
                )
                nc.vector.tensor_scalar_mul(
                    bd23[0:64, 0:64], ctx[0:64, 130:194], rk23[0:64, 0:1]
                )
                nc.vector.tensor_scalar_mul(
                    bd23[64:128, 64:128], ctx[64:128, 194:258], rk23[64:128, 0:1]
                )

            with (
                tc.tile_pool(name="trp", bufs=2, space="PSUM") as trp,
                tc.tile_pool(name="mtp", bufs=2, space="PSUM") as mtp,
            ):
                for pair, bd, mt in ((0, bd01, MT01), (1, bd23, MT23)):
                    tr = trp.tile([C, C], BF16, tag="tr")
                    nc.tensor.transpose(tr, bd, ident)
                    bdt = consts.tile([C, C], BF16, tag=f"bdt{pair}")
                    nc.vector.tensor_copy(bdt, tr)
                    mm = mtp.tile([C, C], F32, tag="mt")
                    nc.tensor.matmul(
                        mm, lhsT=bdt, rhs=wo_s[:, pair, :],
                        start=True, stop=(pair == 1), skip_group_check=True,
                    )
                    if pair == 0:
                        nc.tensor.matmul(
                            mm, lhsT=ones1, rhs=b2_s,
                            start=False, stop=True, skip_group_check=True,
                        )
                    nc.vector.tensor_copy(mt, mm)

            # ---- pass 2 ----
            with tc.tile_pool(name="finp", bufs=4, space="PSUM") as finp:
                for blk in range(NBLK):
                    nsl = slice(blk * NB, (blk + 1) * NB)
                    fin = finp.tile([C, NB], F32, tag="fin")
                    nc.tensor.matmul(
                        fin, lhsT=MT01, rhs=eqn[:, 0, nsl],
                        start=True, stop=False, skip_group_check=True,
                    )
                    nc.tensor.matmul(
                        fin, lhsT=MT23, rhs=eqn[:, 1, nsl],
                        start=False, stop=True, skip_group_check=True,
                    )
                    ob = osbp.tile([C, NB], F32, tag="ob")
                    if blk % 2 == 0:
                        nc.scalar.copy(ob, fin)
                    else:
                        nc.vector.tensor_copy(ob, fin)
                    nc.sync.dma_start(out=out[:, nsl], in_=ob)

    nc.compile()
    return nc


_NC_CACHE = None


def _prep_consts(w_qkv, w_out, b_out):
    bf = ml_dtypes.bfloat16
    wqT = np.ascontiguousarray(np.asarray(w_qkv, dtype=np.float32).T.astype(bf))
    woT = np.ascontiguousarray(np.asarray(w_out, dtype=np.float32).T.astype(bf))
    b2 = np.ascontiguousarray(
        (np.asarray(b_out, dtype=np.float32) / 2.0).reshape(1, C).astype(bf)
    )
    amk = np.zeros((C, C), dtype=np.float32)
    for p in range(C):
        h = p // 64
        amk[p, h * 64:(h + 1) * 64] = 1.0
    amk = np.ascontiguousarray(amk.astype(bf))
    return wqT, woT, b2, amk


def kernel(x, w_qkv, w_out, b_out):
    global _NC_CACHE
    if _NC_CACHE is None:
        _NC_CACHE = build_nc()
    nc = _NC_CACHE

    b = x.shape[0]
    bf = ml_dtypes.bfloat16
    wqT, woT, b2, amk = _prep_consts(w_qkv, w_out, b_out)
    xb = np.asarray(x, dtype=np.float32).reshape(b, C, N).astype(bf)
    in_maps = [
        {"x": np.ascontiguousarray(xb[i]), "wqT": wqT, "woT": woT,
         "b2": b2, "am": amk}
        for i in range(b)
    ]
    res = run_bass_kernel_spmd(nc, in_maps, core_ids=list(range(b)))
    return np.stack(
        [res.results[i]["out"].reshape(C, 128, 128) for i in range(b)]
    ).astype(np.float32)


# revision 8
# speedup vs baseline: 1.6374x; 1.0067x over previous
"""LinearAttention Trainium2 Bass kernel, v3.

Data-parallel over batch (b=8) across 8 NeuronCores. Per core, per
512-pixel block (C=128, n=16384):

  pass 1:  q_ps[hd,n] = wq^T @ x        (2 MMs N=512, layout A)
           kv_ps[n, k|v] = x_s^T @ w_kv (4 MMs N=512, layout B)
           eqA <- exp(q_ps)   [one scalar ACT [128,1024], raw]
           ek  <- exp(k part) [one scalar ACT [128,1024] strided]
           vt  <- cast(v part) [2 DVE casts; ones cols preset]
           ctx01|ctx23 += ek_s^T @ [v01|1] / [1|v23]  (8 MMs N=129, 1 bank)
           per pair p: sb = A^T @ eqA_p  (blockdiag-ones mask matmul ->
             per-head exp-sums broadcast to all 64 partitions of the head),
           rsI = approx(1/sb) [DVE], eqn = eqA_p * rsI [gpsimd, SBUF*SBUF]
           eqn persists (normalized exp q, layout A).

  fold:    MT_pair = blockdiag(ctx/s_k)^T @ woT_pair (+ rank-1 bias/2 on
           pair01; exact since softmax columns sum to 1)

  pass 2:  fin = MT01^T @ eqn01 + MT23^T @ eqn23  (2 MMs N=512)
           out <- DMA(scalar copy of fin)

All matmul operands bf16; PSUM accumulation fp32.
"""

import numpy as np
import ml_dtypes

import concourse.bass as bass
import concourse.tile as tile
from concourse import bacc, mybir
from concourse.bass_utils import run_bass_kernel_spmd
from concourse.masks import make_identity

F32 = mybir.dt.float32
BF16 = mybir.dt.bfloat16
AF = mybir.ActivationFunctionType

C = 128
N = 16384
INNER = 256
NB = 512
SUB = 4
NBLK = N // NB


def build_nc():
    nc = bacc.Bacc("TRN2", target_bir_lowering=False, debug=False, num_devices=8)

    x = nc.dram_tensor("x", [C, N], BF16, kind="ExternalInput")
    wqT = nc.dram_tensor("wqT", [C, 3 * INNER], BF16, kind="ExternalInput")
    woT = nc.dram_tensor("woT", [INNER, C], BF16, kind="ExternalInput")
    b2 = nc.dram_tensor("b2", [1, C], BF16, kind="ExternalInput")
    am = nc.dram_tensor("am", [C, C], BF16, kind="ExternalInput")
    out = nc.dram_tensor("out", [C, N], F32, kind="ExternalOutput")

    with tile.TileContext(nc) as tc:
        with (
            tc.tile_pool(name="consts", bufs=1) as consts,
            tc.tile_pool(name="eqa", bufs=1) as eqa,
            tc.tile_pool(name="xin", bufs=4) as xin,
            tc.tile_pool(name="eqtp", bufs=3) as eqtp,
            tc.tile_pool(name="ekp", bufs=3) as ekp,
            tc.tile_pool(name="vtp", bufs=1) as vtp,
            tc.tile_pool(name="rsp", bufs=3) as rsp,
            tc.tile_pool(name="small", bufs=2) as small,
            tc.tile_pool(name="osb", bufs=4) as osbp,
        ):
            wq_s = consts.tile([C, 3 * INNER], BF16)
            nc.sync.dma_start(out=wq_s, in_=wqT[:, :])
            wo_s = consts.tile([C, 2, C], BF16)
            nc.sync.dma_start(out=wo_s[:, 0, :], in_=woT[0:128, :])
            nc.sync.dma_start(out=wo_s[:, 1, :], in_=woT[128:256, :])
            b2_s = consts.tile([1, C], BF16)
            nc.sync.dma_start(out=b2_s, in_=b2[:, :])
            am_s = consts.tile([C, C], BF16)
            nc.sync.dma_start(out=am_s, in_=am[:, :])
            ones1 = consts.tile([1, C], BF16)
            nc.gpsimd.memset(ones1, 1.0)
            ident = consts.tile([C, C], BF16)
            make_identity(nc, ident)

            eqn = eqa.tile([C, 2, N], BF16)       # normalized exp(q), layout A
            MT01 = consts.tile([C, C], BF16)
            MT23 = consts.tile([C, C], BF16)

            vt2 = vtp.tile([C, 2, SUB, 258], BF16)
            nc.gpsimd.memset(vt2[:, 0, :, 128:130], 1.0)
            nc.gpsimd.memset(vt2[:, 1, :, 128:130], 1.0)

            with (
                tc.tile_pool(name="qp", bufs=1, space="PSUM") as qp,
                tc.tile_pool(name="kvp", bufs=1, space="PSUM") as kvp,
                tc.tile_pool(name="ctxp", bufs=1, space="PSUM") as ctxp,
                tc.tile_pool(name="sbp", bufs=1, space="PSUM") as sbp,
            ):
                ctx = ctxp.tile([C, 512], F32)
                sbF = sbp.tile([C, 2, NB], F32)

                # software-pipelined emission: producers for block `blk` and
                # consumers for block `blk-1` share an iteration, so every
                # PE instruction's dependencies were issued a full slot
                # earlier and the in-order PE queue never stalls mid-stream.
                eqA_t = [None, None]
                ek_t = [None, None]

                for blk in range(NBLK + 1):
                    eqA = None
                    if blk < NBLK:
                        nsl = slice(blk * NB, (blk + 1) * NB)
                        x_blk = xin.tile([C, NB], BF16, tag="x")
                        nc.sync.dma_start(out=x_blk, in_=x[:, nsl])

                        eqA = eqtp.tile([C, 2, NB], BF16, tag="eqA")
                        # q half 0 (1-bank q psum; half 1 emitted later so the
                        # WAR on the bank sits behind kv/ctx work)
                        q_ps = qp.tile([C, 2, NB // 2], F32, tag="q")
                        nc.tensor.matmul(
                            q_ps[:, 0, :], lhsT=wq_s[:, 0:128],
                            rhs=x_blk[:, 0:256],
                            start=True, stop=True, skip_group_check=True,
                        )
                        nc.tensor.matmul(
                            q_ps[:, 1, :], lhsT=wq_s[:, 128:256],
                            rhs=x_blk[:, 0:256],
                            start=True, stop=True, skip_group_check=True,
                        )
                        nc.scalar.activation(eqA[:, :, 0:256], q_ps, AF.Exp)
                        kv_ps = kvp.tile([C, SUB, NB], F32, tag="kv")
                        for s in range(SUB):
                            nc.tensor.matmul(
                                kv_ps[:, s, :],
                                lhsT=x_blk[:, s * 128:(s + 1) * 128],
                                rhs=wq_s[:, 256:768],
                                start=True, stop=True, skip_group_check=True,
                            )

                    if blk >= 1:
                        # consumers of block blk-1 on the PE stream
                        p = blk - 1
                        pek = ek_t[p % 2]
                        pvt = vt2[:, p % 2]
                        peq = eqA_t[p % 2]
                        pnsl = slice(p * NB, (p + 1) * NB)
                        for s in range(SUB):
                            nc.tensor.matmul(
                                ctx[:, 0:129], lhsT=pek[:, s, 0:128],
                                rhs=pvt[:, s, 0:129],
                                start=(p == 0 and s == 0),
                                stop=(p == NBLK - 1 and s == SUB - 1),
                                skip_group_check=True,
                            )
                            nc.tensor.matmul(
                                ctx[:, 129:258], lhsT=pek[:, s, 128:256],
                                rhs=pvt[:, s, 129:258],
                                start=(p == 0 and s == 0),
                                stop=(p == NBLK - 1 and s == SUB - 1),
                                skip_group_check=True,
                            )
                        # both sb matmuls adjacent (2 banks), one approx op
                        nc.tensor.matmul(
                            sbF[:, 0, :], lhsT=am_s, rhs=peq[:, 0, :],
                            start=True, stop=True, skip_group_check=True,
                        )
                        nc.tensor.matmul(
                            sbF[:, 1, :], lhsT=am_s, rhs=peq[:, 1, :],
                            start=True, stop=True, skip_group_check=True,
                        )
                        rsI = rsp.tile([C, 2, NB], F32, tag="rsI")
                        nc.vector.reciprocal_approx_fast(rsI, sbF)
                        nc.gpsimd.tensor_mul(
                            eqn[:, 0, pnsl], peq[:, 0, :], rsI[:, 0, :]
                        )
                        nc.gpsimd.tensor_mul(
                            eqn[:, 1, pnsl], peq[:, 1, :], rsI[:, 1, :]
                        )

                    if blk < NBLK:
                        # q half 1 reuses the q psum bank after the first exp
                        nc.tensor.matmul(
                            q_ps[:, 0, :], lhsT=wq_s[:, 0:128],
                            rhs=x_blk[:, 256:512],
                            start=True, stop=True, skip_group_check=True,
                        )
                        nc.tensor.matmul(
                            q_ps[:, 1, :], lhsT=wq_s[:, 128:256],
                            rhs=x_blk[:, 256:512],
                            start=True, stop=True, skip_group_check=True,
                        )
                        nc.scalar.activation(eqA[:, :, 256:512], q_ps, AF.Exp)
                        ek = ekp.tile([C, SUB, INNER], BF16, tag="ek")
                        nc.scalar.activation(ek, kv_ps[:, :, 0:256], AF.Exp)
                        eqA_t[blk % 2] = eqA
                        ek_t[blk % 2] = ek

                        vt = vt2[:, blk % 2]
                        vo = vt[:, :, 0:128]
                        vout = bass.AP(
                            tensor=vo.tensor, offset=vo.offset,
                            ap=[list(vo.ap[0]), list(vo.ap[1]), [130, 2],
                                [1, 128]],
                        )
                        vi = kv_ps[:, :, 256:384]
                        vin = bass.AP(
                            tensor=vi.tensor, offset=vi.offset,
                            ap=[list(vi.ap[0]), list(vi.ap[1]), [128, 2],
                                [1, 128]],
                        )
                        nc.vector.tensor_copy(vout, vin)

                # ---- fold ----
                rk01 = small.tile([C, 1], F32, tag="rk01")
                rk23 = small.tile([C, 1], F32, tag="rk23")
                nc.vector.reciprocal(rk01, ctx[:, 128:129])
                nc.vector.reciprocal(rk23, ctx[:, 129:130])
                bd01 = consts.tile([C, C], BF16)
                bd23 = consts.tile([C, C], BF16)
                nc.gpsimd.memset(bd01, 0.0)
                nc.gpsimd.memset(bd23, 0.0)
                nc.vector.tensor_scalar_mul(
                    bd01[0:64, 0:64], ctx[0:64, 0:64], rk01[0:64, 0:1]
                )
                nc.vector.tensor_scalar_mul(
                    bd01[64:128, 64:128], ctx[64:128, 64:128], rk01[64:128, 0:1]
                )
                nc.vector.tensor_scalar_mul(
                    bd23[0:64, 0:64], ctx[0:64, 130:194], rk23[0:64, 0:1]
                )
                nc.vector.tensor_scalar_mul(
                    bd23[64:128, 64:128], ctx[64:128, 194:258], rk23[64:128, 0:1]
                )

            with (
                tc.tile_pool(name="trp", bufs=2, space="PSUM") as trp,
                tc.tile_pool(name="mtp", bufs=2, space="PSUM") as mtp,
            ):
                for pair, bd, mt in ((0, bd01, MT01), (1, bd23, MT23)):
                    tr = trp.tile([C, C], BF16, tag="tr")
                    nc.tensor.transpose(tr, bd, ident)
                    bdt = consts.tile([C, C], BF16, tag=f"bdt{pair}")
                    nc.vector.tensor_copy(bdt, tr)
                    mm = mtp.tile([C, C], F32, tag="mt")
                    nc.tensor.matmul(
                        mm, lhsT=bdt, rhs=wo_s[:, pair, :],
                        start=True, stop=(pair == 1), skip_group_check=True,
                    )
                    if pair == 0:
                        nc.tensor.matmul(
                            mm, lhsT=ones1, rhs=b2_s,
                            start=False, stop=True, skip_group_check=True,
                        )
                    nc.vector.tensor_copy(mt, mm)

            # ---- pass 2 ----
            with tc.tile_pool(name="finp", bufs=4, space="PSUM") as finp:
                for blk in range(NBLK):
                    nsl = slice(blk * NB, (blk + 1) * NB)
                    fin = finp.tile([C, NB], F32, tag="fin")
                    nc.tensor.matmul(
                        fin, lhsT=MT01, rhs=eqn[:, 0, nsl],
                        start=True, stop=False, skip_group_check=True,
                    )
                    nc.tensor.matmul(
                        fin, lhsT=MT23, rhs=eqn[:, 1, nsl],
                        start=False, stop=True, skip_group_check=True,
                    )
                    ob = osbp.tile([C, NB], F32, tag="ob")
                    if blk % 2 == 0:
                        nc.scalar.copy(ob, fin)
                    else:
                        nc.vector.tensor_copy(ob, fin)
                    nc.sync.dma_start(out=out[:, nsl], in_=ob)

    nc.compile()
    return nc


_NC_CACHE = None


def _prep_consts(w_qkv, w_out, b_out):
    bf = ml_dtypes.bfloat16
    wqT = np.ascontiguousarray(np.asarray(w_qkv, dtype=np.float32).T.astype(bf))
    woT = np.ascontiguousarray(np.asarray(w_out, dtype=np.float32).T.astype(bf))
    b2 = np.ascontiguousarray(
        (np.asarray(b_out, dtype=np.float32) / 2.0).reshape(1, C).astype(bf)
    )
    amk = np.zeros((C, C), dtype=np.float32)
    for p in range(C):
        h = p // 64
        amk[p, h * 64:(h + 1) * 64] = 1.0
    amk = np.ascontiguousarray(amk.astype(bf))
    return wqT, woT, b2, amk


def kernel(x, w_qkv, w_out, b_out):
    global _NC_CACHE
    if _NC_CACHE is None:
        _NC_CACHE = build_nc()
    nc = _NC_CACHE

    b = x.shape[0]
    bf = ml_dtypes.bfloat16
    wqT, woT, b2, amk = _prep_consts(w_qkv, w_out, b_out)
    xb = np.asarray(x, dtype=np.float32).reshape(b, C, N).astype(bf)
    in_maps = [
        {"x": np.ascontiguousarray(xb[i]), "wqT": wqT, "woT": woT,
         "b2": b2, "am": amk}
        for i in range(b)
    ]
    res = run_bass_kernel_spmd(nc, in_maps, core_ids=list(range(b)))
    return np.stack(
        [res.results[i]["out"].reshape(C, 128, 128) for i in range(b)]
    ).astype(np.float32)
